# revision 1
# baseline (speedup 1.0000x reference)
"""GraphSAGE message-passing kernel for 8 Trainium2 NeuronCores.

reference semantics:
    h = relu(feat @ W0)
    deg = segment_sum(ones, dst); inv = 1/max(deg,1)
    for l in 0..2: h = relu((segment_sum(h[src], dst) * inv) @ Ws[l])
    out = concat([h0, h1, h2, h3], axis=1)          # [50000, 512]

Distribution: nodes sharded by dst range (6250/core, padded to 6272), edges
live on their dst-owner core, sorted by (node-group, src-table-half).  Each
layer: row-gather h[src] from replicated bf16 half-tables in HBM (dma_gather,
int16 idx), segment-sum via one-hot selection matmuls on TensorE accumulating
in PSUM per 128-node group, inv-degree scale on ScalarE, PE transpose, weight
matmul, ReLU.  Per-core h chunks are AllGathered into the next layer's two
half-tables; the first-half AllGather fires mid-layer to overlap compute.
"""
import sys

sys.path.insert(0, "/opt/trn_rl_repo")

import numpy as np
import ml_dtypes

N_NODES = 50000
N_EDGES = 800000
D = 128
NLAYERS = 3
NCORES = 8
NPC = N_NODES // NCORES          # 6250 nodes per core
NGRP = (NPC + 127) // 128        # 49 groups of 128 nodes
NPCP = NGRP * 128                # 6272 padded nodes per core
GA = 25                          # groups in half A (lo table)
GB = NGRP - GA                   # 24 groups in half B (hi table)
NA = GA * 128                    # 3200 nodes per core in half A
NB = GB * 128                    # 3072 (incl. 22 pad rows)
TA = NCORES * NA                 # 25600 lo-table rows (all written by AG)
TB = NCORES * NB                 # 24576 hi-table rows
PAD_LO = 0                       # pads gather row 0; dstrel=-1 zeroes their S column
PAD_HI = 0                       # pads gather row 0; dstrel=-1 zeroes their S column
PIECE_G = 4                      # groups gathered/built per pipeline piece

_RUNTIME = {}


def _patch_tile_drain():
    from concourse import mybir
    from concourse.tile import TileContext, ScopedClock

    if getattr(TileContext, "_drain_patched", False):
        return

    def _drain_and_barrier(self, tick_clock, wait_clock):
        # This walrus build rejects >1 sem-wait on one instruction; split the
        # kernel-tail drain waits across single-wait nops on SP.
        nc = self.nc
        probe = nc.sync.nop()
        wait_clock.add_sem_waits(
            probe.ins, ScopedClock({None: tick_clock.global_clock})
        )
        si = probe.ins.sync_info
        waits = list(si.on_wait) if si is not None else []
        if len(waits) > 1:
            si.on_wait = waits[:1]
            for w in waits[1:]:
                n = nc.sync.nop()
                n.ins.sync_info = mybir.SyncInfo(on_wait=[w], on_update=[])
        nc.sync.drain()
        nc.all_engine_barrier()
        popped = nc._tile_sem_poison_stack.pop()
        assert popped is self._sem_poison
        nc.clear_and_free_semaphores(list(self.sems.allocated().values()))
        nc.all_engine_barrier()

    TileContext._drain_and_barrier = _drain_and_barrier
    TileContext._drain_patched = True


def _pack_idxs(idx):
    """Pack one dma_gather call's index sequence.

    Slot L of the gather output sits at partition L%128, free slot L//128;
    the Q7 kernel reads the index for that slot from wrapped[p%16, p//16+8*s].
    Returns [16, n/16] int16 (caller concatenates calls and tiles to 128).
    """
    idx = np.asarray(idx, dtype=np.int16)
    n = len(idx)
    assert n % 128 == 0
    L = np.arange(n)
    s, p = L // 128, L % 128
    wrapped = np.zeros((16, n // 16), dtype=np.int16)
    wrapped[p % 16, p // 16 + 8 * s] = idx
    return wrapped


def _prepare(feat, src, dst):
    """Host-side sharding/sorting/padding. Returns per-core tensors + plan."""
    src = np.asarray(src).astype(np.int64)
    dst = np.asarray(dst).astype(np.int64)
    feat = np.asarray(feat, dtype=np.float32)

    deg = np.bincount(dst, minlength=N_NODES).astype(np.float32)
    invdeg = (1.0 / np.maximum(deg, 1.0)).astype(np.float32)

    owner = dst // NPC
    dstl = dst - owner * NPC
    sc = src // NPC
    sj = src - sc * NPC                           # src local node 0..6249
    hi = sj >= NA                                 # table half by src local id
    srcr = np.where(hi, NB * sc + (sj - NA), NA * sc + sj)
    group = dstl >> 7

    counts = np.zeros((NCORES, NGRP, 2), dtype=np.int64)
    per_core = []
    per_core_iv = []
    for c in range(NCORES):
        m = owner == c
        key = group[m] * 2 + hi[m]
        order = np.argsort(key, kind="stable")
        e_srcr = srcr[m][order]
        e_dstrel = (dstl[m] - (group[m] << 7))[order]
        e_key = key[order]
        cnt = np.bincount(e_key, minlength=NGRP * 2).reshape(NGRP, 2)
        counts[c] = cnt
        per_core.append((e_srcr, e_dstrel, e_key))
        per_core_iv.append(invdeg[dst[m]][order])

    # uniform chunk plan: chunks of 128 edges, count = max over cores
    mx = counts.max(axis=0)                       # [NGRP, 2]
    NLO = np.maximum((mx[:, 0] + 127) // 128, 1).astype(np.int64)
    NHI = np.maximum((mx[:, 1] + 127) // 128, 1).astype(np.int64)

    pieces = [
        list(range(p0, min(p0 + PIECE_G, NGRP))) for p0 in range(0, NGRP, PIECE_G)
    ]

    tot_lo = int(NLO.sum()) * 128
    tot_hi = int(NHI.sum()) * 128
    totch = int((NLO + NHI).sum())

    idx_lo = np.zeros((NCORES, 128, tot_lo // 16), dtype=np.int16)
    idx_hi = np.zeros((NCORES, 128, tot_hi // 16), dtype=np.int16)
    dstrel = np.zeros((NCORES, 128, totch), dtype=ml_dtypes.bfloat16)
    ivedge = np.zeros((NCORES, 128, totch), dtype=ml_dtypes.bfloat16)
    featT = np.zeros((NCORES, D, NPCP), dtype=np.float32)

    for c in range(NCORES):
        e_srcr, e_dstrel, e_key = per_core[c]
        e_iv = per_core_iv[c]
        starts = np.zeros(NGRP * 2 + 1, dtype=np.int64)
        np.cumsum(np.bincount(e_key, minlength=NGRP * 2), out=starts[1:])

        lo_seq, hi_seq = [], []
        ch = 0
        for g in range(NGRP):
            for s, (seq, NC_, padv) in (
                (0, (lo_seq, NLO, PAD_LO)),
                (1, (hi_seq, NHI, PAD_HI)),
            ):
                a, b = starts[g * 2 + s], starts[g * 2 + s + 1]
                n_pad = int(NC_[g]) * 128
                ids = np.full(n_pad, padv, dtype=np.int64)
                ids[: b - a] = e_srcr[a:b]
                seq.append(ids)
                dr = np.full(n_pad, -1.0, dtype=np.float32)
                dr[: b - a] = e_dstrel[a:b]
                iv2 = np.zeros(n_pad, dtype=np.float32)
                iv2[: b - a] = e_iv[a:b]
                nch = n_pad // 128
                dstrel[c, :, ch : ch + nch] = (
                    dr.reshape(nch, 128).T.astype(ml_dtypes.bfloat16)
                )
                ivedge[c, :, ch : ch + nch] = (
                    iv2.reshape(nch, 128).T.astype(ml_dtypes.bfloat16)
                )
                ch += nch
        assert ch == totch

        lo_seq = np.concatenate(lo_seq)
        hi_seq = np.concatenate(hi_seq)
        lo_off = np.concatenate(([0], np.cumsum(NLO) * 128))
        hi_off = np.concatenate(([0], np.cumsum(NHI) * 128))
        lo_blocks, hi_blocks = [], []
        for grp in pieces:
            g0, g1 = grp[0], grp[-1] + 1
            lo_blocks.append(_pack_idxs(lo_seq[lo_off[g0] : lo_off[g1]]))
            hi_blocks.append(_pack_idxs(hi_seq[hi_off[g0] : hi_off[g1]]))
        idx_lo[c] = np.tile(np.concatenate(lo_blocks, axis=1), (8, 1))
        idx_hi[c] = np.tile(np.concatenate(hi_blocks, axis=1), (8, 1))

        featT[c, :, :NPC] = feat[c * NPC : (c + 1) * NPC].T

    plan = {
        "NLO": NLO.tolist(),
        "NHI": NHI.tolist(),
        "pieces": pieces,
        "tot_lo": tot_lo,
        "tot_hi": tot_hi,
        "totch": totch,
    }
    slabs = {
        "idx_lo": idx_lo,
        "idx_hi": idx_hi,
        "dstrel": dstrel,
        "ivedge": ivedge,
        "featT": featT,
    }
    return plan, slabs


def _build(plan, ablate=()):
    from concourse import mybir, tile, bacc

    _patch_tile_drain()

    NLO, NHI = plan["NLO"], plan["NHI"]
    pieces = plan["pieces"]
    tot_lo, tot_hi, totch = plan["tot_lo"], plan["tot_hi"], plan["totch"]
    bf16, f32, i16 = mybir.dt.bfloat16, mybir.dt.float32, mybir.dt.int16

    nc = bacc.Bacc("TRN2")
    p_idx_lo = nc.declare_dram_parameter("idx_lo", [128, tot_lo // 16], i16, isOutput=False)
    p_idx_hi = nc.declare_dram_parameter("idx_hi", [128, tot_hi // 16], i16, isOutput=False)
    p_dstrel = nc.declare_dram_parameter("dstrel", [128, totch], bf16, isOutput=False)
    p_ivedge = nc.declare_dram_parameter("ivedge", [128, totch], bf16, isOutput=False)
    p_featT = nc.declare_dram_parameter("featT", [D, NPCP], f32, isOutput=False)
    p_W0 = nc.declare_dram_parameter("W0", [D, D], f32, isOutput=False)
    p_Ws = nc.declare_dram_parameter("Ws", [D, NLAYERS, D], bf16, isOutput=False)
    p_out = nc.declare_dram_parameter("out", [NPCP, (NLAYERS + 1) * D], f32, isOutput=True)

    iota_np = np.tile(np.arange(128, dtype=ml_dtypes.bfloat16)[None, :], (128, 1))
    eye_np = np.eye(128, dtype=ml_dtypes.bfloat16)

    ch_off = [0]
    for g in range(NGRP):
        ch_off.append(ch_off[-1] + NLO[g] + NHI[g])
    lo_choff = np.concatenate(([0], np.cumsum(NLO)))
    hi_choff = np.concatenate(([0], np.cumsum(NHI)))

    with tile.TileContext(nc) as tc:
        iota_d = nc.inline_tensor(iota_np, name="iota_c")
        with (
            tc.tile_pool(name="const", bufs=1) as cpool,
            tc.tile_pool(name="glo", bufs=2) as glo_pool,
            tc.tile_pool(name="ghi", bufs=2) as ghi_pool,
            tc.tile_pool(name="sel", bufs=2) as sel_pool,
            tc.tile_pool(name="small", bufs=3) as small,
            tc.tile_pool(name="psA", bufs=2, space="PSUM") as psA,
            tc.tile_pool(name="psC", bufs=2, space="PSUM") as psC,
            tc.tile_pool(name="dram", bufs=1, space="DRAM") as dram,
        ):
            iota = cpool.tile([128, 128], bf16)
            nc.sync.dma_start(out=iota[:], in_=iota_d[:, :])
            W0 = cpool.tile([D, D], f32)
            nc.sync.dma_start(out=W0[:], in_=p_W0[:, :])
            Ws = cpool.tile([D, NLAYERS, D], bf16)
            nc.sync.dma_start(out=Ws[:], in_=p_Ws[:, :, :])
            ivedge = cpool.tile([128, totch], bf16)
            nc.sync.dma_start(out=ivedge[:], in_=p_ivedge[:, :])
            featT = cpool.tile([D, NPCP], f32)
            nc.sync.dma_start(out=featT[:], in_=p_featT[:, :])
            idxlo = cpool.tile([128, tot_lo // 16], i16)
            nc.sync.dma_start(out=idxlo[:], in_=p_idx_lo[:, :])
            idxhi = cpool.tile([128, tot_hi // 16], i16)
            nc.sync.dma_start(out=idxhi[:], in_=p_idx_hi[:, :])
            dstrel = cpool.tile([128, totch], bf16)
            nc.sync.dma_start(out=dstrel[:], in_=p_dstrel[:, :])

            ag_a = [
                dram.tile([NA, D], bf16, tag=f"aga{i}", name=f"aga{i}")
                for i in range(NLAYERS)
            ]
            ag_b = [
                dram.tile([NB, D], bf16, tag=f"agb{i}", name=f"agb{i}")
                for i in range(NLAYERS)
            ]
            tab_lo = [
                dram.tile(
                    [TA, D], bf16, addr_space="Shared",
                    tag=f"tlo{i}", name=f"tlo{i}",
                )
                for i in range(NLAYERS)
            ]
            tab_hi = [
                dram.tile(
                    [TB, D], bf16, addr_space="Shared",
                    tag=f"thi{i}", name=f"thi{i}",
                )
                for i in range(NLAYERS)
            ]
            def ag_half(layer, half):
                if "cc" in ablate:
                    return
                src_t = (ag_a if half == 0 else ag_b)[layer]
                dst_t = (tab_lo if half == 0 else tab_hi)[layer]
                n = TA if half == 0 else TB
                nc.gpsimd.collective_compute(
                    "AllGather", mybir.AluOpType.bypass,
                    replica_groups=[list(range(NCORES))],
                    ins=[src_t[:]], outs=[dst_t[:n, :]],
                )

            def emit_h_block(b, h_ps, layer):
                """Evacuate one [128 nodes, 128] psum block: relu -> out cols,
                bf16 copy -> ag buffers (except last layer)."""
                r0 = b * 128
                h_f = small.tile([128, D], f32, tag="hf")
                nc.scalar.activation(
                    out=h_f[:], in_=h_ps[:],
                    func=mybir.ActivationFunctionType.Relu,
                )
                nc.sync.dma_start(
                    out=p_out[r0 : r0 + 128, layer * D : (layer + 1) * D],
                    in_=h_f[:],
                )
                if layer < NLAYERS:
                    h_b = small.tile([128, D], bf16, tag="hb")
                    nc.vector.tensor_copy(out=h_b[:], in_=h_f[:])
                    if b < GA:
                        nc.sync.dma_start(
                            out=ag_a[layer][r0 : r0 + 128, :], in_=h_b[:]
                        )
                    else:
                        rb = (b - GA) * 128
                        nc.sync.dma_start(
                            out=ag_b[layer][rb : rb + 128, :], in_=h_b[:]
                        )
                    if b == GA - 1:
                        ag_half(layer, 0)
                    elif b == NGRP - 1:
                        ag_half(layer, 1)

            # ---- phase 0: h0 = relu(feat @ W0) ----
            for b in range(NGRP):
                h_ps = psC.tile([128, D], f32, space="PSUM", tag="hps")
                nc.tensor.matmul(
                    out=h_ps[:], lhsT=featT[:, b * 128 : (b + 1) * 128],
                    rhs=W0[:], start=True, stop=True,
                )
                emit_h_block(b, h_ps, 0)

            # ---- layers ----
            for l in range(NLAYERS):
                for pi, grp in enumerate(pieces):
                    g0, g1 = grp[0], grp[-1] + 1
                    plo = int(lo_choff[g1] - lo_choff[g0])
                    phi = int(hi_choff[g1] - hi_choff[g0])
                    G_lo = glo_pool.tile([128, plo, D], bf16, tag="glo")
                    G_hi = ghi_pool.tile([128, phi, D], bf16, tag="ghi")
                    if l == 0 and pi < 2:
                        # first use of each pool slot: stale SBUF bits could
                        # be NaN; padded slots must stay finite (S column is
                        # zero for pads, but 0*NaN still poisons PSUM)
                        nc.vector.memset(G_lo[:], 0)
                        nc.vector.memset(G_hi[:], 0)
                    if "gather" in ablate:
                        nc.vector.memset(G_lo[:], 0)
                        nc.vector.memset(G_hi[:], 0)
                    else:
                        nc.gpsimd.dma_gather(
                            G_lo[:], tab_lo[l][:, :],
                            idxlo[:, lo_choff[g0] * 8 : lo_choff[g1] * 8],
                            plo * 128, plo * 128, D, single_packet=False,
                        )
                        nc.gpsimd.dma_gather(
                            G_hi[:], tab_hi[l][:, :],
                            idxhi[:, hi_choff[g0] * 8 : hi_choff[g1] * 8],
                            phi * 128, phi * 128, D, single_packet=False,
                        )
                    nch = ch_off[g1] - ch_off[g0]
                    S = sel_pool.tile([128, nch, 128], bf16, tag="sel")
                    nc.vector.tensor_tensor(
                        out=S[:],
                        in0=dstrel[:, ch_off[g0] : ch_off[g1]][:, :, None]
                        .to_broadcast([128, nch, 128]),
                        in1=iota[:][:, None, :].to_broadcast([128, nch, 128]),
                        op=mybir.AluOpType.is_equal,
                    )
                    nc.vector.tensor_tensor(
                        out=S[:],
                        in0=S[:],
                        in1=ivedge[:, ch_off[g0] : ch_off[g1]][:, :, None]
                        .to_broadcast([128, nch, 128]),
                        op=mybir.AluOpType.mult,
                    )
                    for g in grp:
                        aggT_ps = psA.tile([128, D], f32, space="PSUM", tag="aggT")
                        nmm = NLO[g] + NHI[g]
                        if "seg" in ablate:
                            nmm = 1
                        k = 0
                        for j in range(NLO[g] if "seg" not in ablate else 1):
                            nc.tensor.matmul(
                                out=aggT_ps[:],
                                lhsT=G_lo[:, int(lo_choff[g] - lo_choff[g0]) + j, :],
                                rhs=S[:, ch_off[g] - ch_off[g0] + k, :],
                                start=(k == 0), stop=(k == nmm - 1),
                            )
                            k += 1
                        for j in range(NHI[g] if "seg" not in ablate else 0):
                            nc.tensor.matmul(
                                out=aggT_ps[:],
                                lhsT=G_hi[:, int(hi_choff[g] - hi_choff[g0]) + j, :],
                                rhs=S[:, ch_off[g] - ch_off[g0] + k, :],
                                start=(k == 0), stop=(k == nmm - 1),
                            )
                            k += 1
                        aggT = small.tile([128, D], bf16, tag="aggTsb")
                        nc.vector.tensor_copy(out=aggT[:], in_=aggT_ps[:])
                        h_ps = psC.tile([128, D], f32, space="PSUM", tag="hps")
                        nc.tensor.matmul(
                            out=h_ps[:], lhsT=aggT[:], rhs=Ws[:, l, :],
                            start=True, stop=True,
                        )
                        emit_h_block(g, h_ps, l + 1)
    nc.compile()
    return nc


def kernel(feat, src, dst, W0, Ws):
    from concourse.bass_utils import run_bass_kernel_spmd

    plan, slabs = _prepare(feat, src, dst)
    nc = _build(plan)

    W0_np = np.asarray(W0, dtype=np.float32)
    Ws_np = np.transpose(np.asarray(Ws, dtype=np.float32), (1, 0, 2)).astype(
        ml_dtypes.bfloat16
    )  # [fi, layer, fo]
    in_maps = [
        {
            "idx_lo": slabs["idx_lo"][c],
            "idx_hi": slabs["idx_hi"][c],
            "dstrel": slabs["dstrel"][c],
            "ivedge": slabs["ivedge"][c],
            "featT": slabs["featT"][c],
            "W0": W0_np,
            "Ws": Ws_np,
        }
        for c in range(NCORES)
    ]
    res = None
    last_err = None
    for attempt in range(3):
        try:
            res = run_bass_kernel_spmd(nc, in_maps, core_ids=list(range(NCORES)))
            break
        except Exception as e:  # transient device hiccups (axon RPC, NRT recovery)
            last_err = e
            import time as _time

            _time.sleep(5)
    if res is None:
        raise last_err
    _RUNTIME["nc"] = nc
    _RUNTIME["in_maps"] = in_maps

    out = np.empty((N_NODES, (NLAYERS + 1) * D), dtype=np.float32)
    for c in range(NCORES):
        out[c * NPC : (c + 1) * NPC] = res.results[c]["out"][:NPC]
    return out



# revision 9
# speedup vs baseline: 1.5511x; 1.5511x over previous
"""GraphSAGE message-passing kernel for 8 Trainium2 NeuronCores.

reference semantics:
    h = relu(feat @ W0)
    deg = segment_sum(ones, dst); inv = 1/max(deg,1)
    for l in 0..2: h = relu((segment_sum(h[src], dst) * inv) @ Ws[l])
    out = concat([h0, h1, h2, h3], axis=1)          # [50000, 512]

Distribution: nodes sharded by dst range (6250/core, padded to 6272), edges
live on their dst-owner core, sorted by (node-group, src-table-half).  Each
layer: row-gather h[src] from replicated bf16 half-tables in HBM (dma_gather,
int16 idx), segment-sum via one-hot selection matmuls on TensorE accumulating
in PSUM per 128-node group, inv-degree scale on ScalarE, PE transpose, weight
matmul, ReLU.  Per-core h chunks are AllGathered into the next layer's two
half-tables; the first-half AllGather fires mid-layer to overlap compute.
"""
import sys

sys.path.insert(0, "/opt/trn_rl_repo")

import numpy as np
import ml_dtypes

N_NODES = 50000
N_EDGES = 800000
D = 128
NLAYERS = 3
NCORES = 8
NPC = N_NODES // NCORES          # 6250 nodes per core
NGRP = (NPC + 127) // 128        # 49 groups of 128 nodes
NPCP = NGRP * 128                # 6272 padded nodes per core
GA = 25                          # groups in half A (lo table)
GB = NGRP - GA                   # 24 groups in half B (hi table)
NA = GA * 128                    # 3200 nodes per core in half A
NB = GB * 128                    # 3072 (incl. 22 pad rows)
TA = NCORES * NA                 # 25600 lo-table rows (all written by AG)
TB = NCORES * NB                 # 24576 hi-table rows
PAD_LO = 0                       # pads gather row 0; dstrel=-1 zeroes their S column
PAD_HI = 0                       # pads gather row 0; dstrel=-1 zeroes their S column
PIECE_G = 4                      # groups gathered/built per pipeline piece

_RUNTIME = {}


def _patch_tile_drain():
    from concourse import mybir
    from concourse.tile import TileContext, ScopedClock

    if getattr(TileContext, "_drain_patched", False):
        return

    def _drain_and_barrier(self, tick_clock, wait_clock):
        # This walrus build rejects >1 sem-wait on one instruction; split the
        # kernel-tail drain waits across single-wait nops on SP.
        nc = self.nc
        probe = nc.sync.nop()
        wait_clock.add_sem_waits(
            probe.ins, ScopedClock({None: tick_clock.global_clock})
        )
        si = probe.ins.sync_info
        waits = list(si.on_wait) if si is not None else []
        if len(waits) > 1:
            si.on_wait = waits[:1]
            for w in waits[1:]:
                n = nc.sync.nop()
                n.ins.sync_info = mybir.SyncInfo(on_wait=[w], on_update=[])
        nc.sync.drain()
        nc.all_engine_barrier()
        popped = nc._tile_sem_poison_stack.pop()
        assert popped is self._sem_poison
        nc.clear_and_free_semaphores(list(self.sems.allocated().values()))
        nc.all_engine_barrier()

    TileContext._drain_and_barrier = _drain_and_barrier
    TileContext._drain_patched = True


def _pack_idxs(idx):
    """Pack one dma_gather call's index sequence.

    Slot L of the gather output sits at partition L%128, free slot L//128;
    the Q7 kernel reads the index for that slot from wrapped[p%16, p//16+8*s].
    Returns [16, n/16] int16 (caller concatenates calls and tiles to 128).
    """
    idx = np.asarray(idx, dtype=np.int16)
    n = len(idx)
    assert n % 128 == 0
    L = np.arange(n)
    s, p = L // 128, L % 128
    wrapped = np.zeros((16, n // 16), dtype=np.int16)
    wrapped[p % 16, p // 16 + 8 * s] = idx
    return wrapped


def _prepare(feat, src, dst):
    """Host-side sharding/sorting/padding. Returns per-core tensors + plan."""
    src = np.asarray(src).astype(np.int64)
    dst = np.asarray(dst).astype(np.int64)
    feat = np.asarray(feat, dtype=np.float32)

    deg = np.bincount(dst, minlength=N_NODES).astype(np.float32)
    invdeg = (1.0 / np.maximum(deg, 1.0)).astype(np.float32)

    owner = dst // NPC
    dstl = dst - owner * NPC
    sc = src // NPC
    sj = src - sc * NPC                           # src local node 0..6249
    hi = sj >= NA                                 # table half by src local id
    srcr = np.where(hi, NB * sc + (sj - NA), NA * sc + sj)
    group = dstl >> 7

    counts = np.zeros((NCORES, NGRP, 2), dtype=np.int64)
    per_core = []
    per_core_iv = []
    for c in range(NCORES):
        m = owner == c
        key = group[m] * 2 + hi[m]
        order = np.argsort(key, kind="stable")
        e_srcr = srcr[m][order]
        e_dstrel = (dstl[m] - (group[m] << 7))[order]
        e_key = key[order]
        cnt = np.bincount(e_key, minlength=NGRP * 2).reshape(NGRP, 2)
        counts[c] = cnt
        per_core.append((e_srcr, e_dstrel, e_key))
        per_core_iv.append(invdeg[dst[m]][order])

    # uniform chunk plan: chunks of 128 edges, count = max over cores
    mx = counts.max(axis=0)                       # [NGRP, 2]
    NLO = np.maximum((mx[:, 0] + 127) // 128, 1).astype(np.int64)
    NHI = np.maximum((mx[:, 1] + 127) // 128, 1).astype(np.int64)

    pieces = [
        list(range(p0, min(p0 + PIECE_G, NGRP))) for p0 in range(0, NGRP, PIECE_G)
    ]

    tot_lo = int(NLO.sum()) * 128
    tot_hi = int(NHI.sum()) * 128
    totch = int((NLO + NHI).sum())

    idx_lo = np.zeros((NCORES, 128, tot_lo // 16), dtype=np.int16)
    idx_hi = np.zeros((NCORES, 128, tot_hi // 16), dtype=np.int16)
    dstrel = np.zeros((NCORES, 128, totch), dtype=ml_dtypes.bfloat16)
    ivedge = np.zeros((NCORES, 128, totch), dtype=ml_dtypes.bfloat16)
    featT = np.zeros((NCORES, D, NPCP), dtype=np.float32)

    for c in range(NCORES):
        e_srcr, e_dstrel, e_key = per_core[c]
        e_iv = per_core_iv[c]
        starts = np.zeros(NGRP * 2 + 1, dtype=np.int64)
        np.cumsum(np.bincount(e_key, minlength=NGRP * 2), out=starts[1:])

        lo_seq, hi_seq = [], []
        ch = 0
        for g in range(NGRP):
            for s, (seq, NC_, padv) in (
                (0, (lo_seq, NLO, PAD_LO)),
                (1, (hi_seq, NHI, PAD_HI)),
            ):
                a, b = starts[g * 2 + s], starts[g * 2 + s + 1]
                n_pad = int(NC_[g]) * 128
                ids = np.full(n_pad, padv, dtype=np.int64)
                ids[: b - a] = e_srcr[a:b]
                seq.append(ids)
                dr = np.full(n_pad, -1.0, dtype=np.float32)
                dr[: b - a] = e_dstrel[a:b]
                iv2 = np.zeros(n_pad, dtype=np.float32)
                iv2[: b - a] = e_iv[a:b]
                nch = n_pad // 128
                dstrel[c, :, ch : ch + nch] = (
                    dr.reshape(nch, 128).T.astype(ml_dtypes.bfloat16)
                )
                ivedge[c, :, ch : ch + nch] = (
                    iv2.reshape(nch, 128).T.astype(ml_dtypes.bfloat16)
                )
                ch += nch
        assert ch == totch

        lo_seq = np.concatenate(lo_seq)
        hi_seq = np.concatenate(hi_seq)
        lo_off = np.concatenate(([0], np.cumsum(NLO) * 128))
        hi_off = np.concatenate(([0], np.cumsum(NHI) * 128))
        lo_blocks, hi_blocks = [], []
        for grp in pieces:
            g0, g1 = grp[0], grp[-1] + 1
            lo_blocks.append(_pack_idxs(lo_seq[lo_off[g0] : lo_off[g1]]))
            hi_blocks.append(_pack_idxs(hi_seq[hi_off[g0] : hi_off[g1]]))
        idx_lo[c] = np.tile(np.concatenate(lo_blocks, axis=1), (8, 1))
        idx_hi[c] = np.tile(np.concatenate(hi_blocks, axis=1), (8, 1))

        featT[c, :, :NPC] = feat[c * NPC : (c + 1) * NPC].T

    plan = {
        "NLO": NLO.tolist(),
        "NHI": NHI.tolist(),
        "pieces": pieces,
        "tot_lo": tot_lo,
        "tot_hi": tot_hi,
        "totch": totch,
    }
    slabs = {
        "idx_lo": idx_lo,
        "idx_hi": idx_hi,
        "dstrel": dstrel,
        "ivedge": ivedge,
        "featT": featT,
    }
    return plan, slabs


def _build(plan, ablate=()):
    from concourse import mybir, tile, bacc

    _patch_tile_drain()

    NLO, NHI = plan["NLO"], plan["NHI"]
    pieces = plan["pieces"]
    tot_lo, tot_hi, totch = plan["tot_lo"], plan["tot_hi"], plan["totch"]
    bf16, f32, i16 = mybir.dt.bfloat16, mybir.dt.float32, mybir.dt.int16

    nc = bacc.Bacc("TRN2", num_swdge_queues=4)
    p_idx_lo = nc.declare_dram_parameter("idx_lo", [128, tot_lo // 16], i16, isOutput=False)
    p_idx_hi = nc.declare_dram_parameter("idx_hi", [128, tot_hi // 16], i16, isOutput=False)
    p_dstrel = nc.declare_dram_parameter("dstrel", [128, totch], bf16, isOutput=False)
    p_ivedge = nc.declare_dram_parameter("ivedge", [128, totch], bf16, isOutput=False)
    p_featT = nc.declare_dram_parameter("featT", [D, NPCP], f32, isOutput=False)
    p_W0 = nc.declare_dram_parameter("W0", [D, D], f32, isOutput=False)
    p_Ws = nc.declare_dram_parameter("Ws", [D, NLAYERS, D], bf16, isOutput=False)
    p_out = nc.declare_dram_parameter("out", [NPCP, (NLAYERS + 1) * D], f32, isOutput=True)

    iota_np = np.tile(np.arange(128, dtype=ml_dtypes.bfloat16)[None, :], (128, 1))
    eye_np = np.eye(128, dtype=ml_dtypes.bfloat16)

    ch_off = [0]
    for g in range(NGRP):
        ch_off.append(ch_off[-1] + NLO[g] + NHI[g])
    lo_choff = np.concatenate(([0], np.cumsum(NLO)))
    hi_choff = np.concatenate(([0], np.cumsum(NHI)))

    with tile.TileContext(nc) as tc:
        iota_d = nc.inline_tensor(iota_np, name="iota_c")
        with (
            tc.tile_pool(name="const", bufs=1) as cpool,
            tc.tile_pool(name="glo", bufs=4) as glo_pool,
            tc.tile_pool(name="ghi", bufs=4) as ghi_pool,
            tc.tile_pool(name="sel", bufs=2) as sel_pool,
            tc.tile_pool(name="small", bufs=3) as small,
            tc.tile_pool(name="psA", bufs=2, space="PSUM") as psA,
            tc.tile_pool(name="psC", bufs=2, space="PSUM") as psC,
            tc.tile_pool(name="dram", bufs=1, space="DRAM") as dram,
        ):
            iota = cpool.tile([128, 128], bf16)
            nc.sync.dma_start(out=iota[:], in_=iota_d[:, :])
            W0 = cpool.tile([D, D], f32)
            nc.sync.dma_start(out=W0[:], in_=p_W0[:, :])
            Ws = cpool.tile([D, NLAYERS, D], bf16)
            nc.sync.dma_start(out=Ws[:], in_=p_Ws[:, :, :])
            ivedge = cpool.tile([128, totch], bf16)
            nc.sync.dma_start(out=ivedge[:], in_=p_ivedge[:, :])
            featT = cpool.tile([D, NPCP], f32)
            nc.sync.dma_start(out=featT[:], in_=p_featT[:, :])
            idxlo = cpool.tile([128, tot_lo // 16], i16)
            nc.sync.dma_start(out=idxlo[:], in_=p_idx_lo[:, :])
            idxhi = cpool.tile([128, tot_hi // 16], i16)
            nc.sync.dma_start(out=idxhi[:], in_=p_idx_hi[:, :])
            dstrel = cpool.tile([128, totch], bf16)
            nc.sync.dma_start(out=dstrel[:], in_=p_dstrel[:, :])

            ag_a = [
                dram.tile([NA, D], bf16, tag=f"aga{i}", name=f"aga{i}")
                for i in range(NLAYERS)
            ]
            ag_b = [
                dram.tile([NB, D], bf16, tag=f"agb{i}", name=f"agb{i}")
                for i in range(NLAYERS)
            ]
            tab_lo = [
                dram.tile(
                    [TA, D], bf16, addr_space="Shared",
                    tag=f"tlo{i}", name=f"tlo{i}",
                )
                for i in range(NLAYERS)
            ]
            tab_hi = [
                dram.tile(
                    [TB, D], bf16, addr_space="Shared",
                    tag=f"thi{i}", name=f"thi{i}",
                )
                for i in range(NLAYERS)
            ]
            def ag_half(layer, half):
                if "cc" in ablate:
                    return
                src_t = (ag_a if half == 0 else ag_b)[layer]
                dst_t = (tab_lo if half == 0 else tab_hi)[layer]
                n = TA if half == 0 else TB
                nc.gpsimd.collective_compute(
                    "AllGather", mybir.AluOpType.bypass,
                    replica_groups=[list(range(NCORES))],
                    ins=[src_t[:]], outs=[dst_t[:n, :]],
                )

            def emit_h_block(b, h_ps, layer):
                """Evacuate one [128 nodes, 128] psum block: relu -> out cols,
                bf16 copy -> ag buffers (except last layer)."""
                r0 = b * 128
                h_f = small.tile([128, D], f32, tag="hf")
                nc.scalar.activation(
                    out=h_f[:], in_=h_ps[:],
                    func=mybir.ActivationFunctionType.Relu,
                )
                if "out" not in ablate:
                    nc.sync.dma_start(
                        out=p_out[r0 : r0 + 128, layer * D : (layer + 1) * D],
                        in_=h_f[:],
                    )
                if layer < NLAYERS:
                    h_b = small.tile([128, D], bf16, tag="hb")
                    nc.vector.tensor_copy(out=h_b[:], in_=h_f[:])
                    if b < GA:
                        nc.sync.dma_start(
                            out=ag_a[layer][r0 : r0 + 128, :], in_=h_b[:]
                        )
                    else:
                        rb = (b - GA) * 128
                        nc.sync.dma_start(
                            out=ag_b[layer][rb : rb + 128, :], in_=h_b[:]
                        )
                    if b == GA - 1:
                        ag_half(layer, 0)
                    elif b == NGRP - 1:
                        ag_half(layer, 1)

            # ---- phase 0: h0 = relu(feat @ W0) ----
            for b in range(NGRP):
                h_ps = psC.tile([128, D], f32, space="PSUM", tag="hps")
                nc.tensor.matmul(
                    out=h_ps[:], lhsT=featT[:, b * 128 : (b + 1) * 128],
                    rhs=W0[:], start=True, stop=True,
                )
                emit_h_block(b, h_ps, 0)

            # ---- layers ----
            for l in range(NLAYERS):
                for pi, grp in enumerate(pieces):
                    g0, g1 = grp[0], grp[-1] + 1
                    plo = int(lo_choff[g1] - lo_choff[g0])
                    phi = int(hi_choff[g1] - hi_choff[g0])
                    G_lo = glo_pool.tile([128, plo, D], bf16, tag="glo")
                    G_hi = ghi_pool.tile([128, phi, D], bf16, tag="ghi")
                    if l == 0 and pi < 4:
                        # first use of each pool slot: stale SBUF bits could
                        # be NaN; padded slots must stay finite (S column is
                        # zero for pads, but 0*NaN still poisons PSUM)
                        nc.vector.memset(G_lo[:], 0)
                        nc.vector.memset(G_hi[:], 0)
                    if "gather" in ablate:
                        nc.vector.memset(G_lo[:], 0)
                        nc.vector.memset(G_hi[:], 0)
                    else:
                        nc.gpsimd.dma_gather(
                            G_lo[:], tab_lo[l][:, :],
                            idxlo[:, lo_choff[g0] * 8 : lo_choff[g1] * 8],
                            plo * 128, plo * 128, D, single_packet=False,
                            queue_num=(2 * pi) % 4,
                        )
                        nc.gpsimd.dma_gather(
                            G_hi[:], tab_hi[l][:, :],
                            idxhi[:, hi_choff[g0] * 8 : hi_choff[g1] * 8],
                            phi * 128, phi * 128, D, single_packet=False,
                            queue_num=(2 * pi + 1) % 4,
                        )
                    nch = ch_off[g1] - ch_off[g0]
                    S = sel_pool.tile([128, nch, 128], bf16, tag="sel")
                    if "sbuild" in ablate:
                        # tiny write so the Tile framework sees S allocated;
                        # garbage contents are fine for timing-only ablation
                        nc.vector.memset(S[:, 0:1, :], 0)
                    else:
                        nc.vector.tensor_tensor(
                            out=S[:],
                            in0=dstrel[:, ch_off[g0] : ch_off[g1]][:, :, None]
                            .to_broadcast([128, nch, 128]),
                            in1=iota[:][:, None, :].to_broadcast([128, nch, 128]),
                            op=mybir.AluOpType.is_equal,
                        )
                        nc.vector.tensor_tensor(
                            out=S[:],
                            in0=S[:],
                            in1=ivedge[:, ch_off[g0] : ch_off[g1]][:, :, None]
                            .to_broadcast([128, nch, 128]),
                            op=mybir.AluOpType.mult,
                        )
                    for g in grp:
                        aggT_ps = psA.tile([128, D], f32, space="PSUM", tag="aggT")
                        nmm = NLO[g] + NHI[g]
                        if "seg" in ablate:
                            nmm = 1
                        k = 0
                        for j in range(NLO[g] if "seg" not in ablate else 1):
                            nc.tensor.matmul(
                                out=aggT_ps[:],
                                lhsT=G_lo[:, int(lo_choff[g] - lo_choff[g0]) + j, :],
                                rhs=S[:, ch_off[g] - ch_off[g0] + k, :],
                                start=(k == 0), stop=(k == nmm - 1),
                            )
                            k += 1
                        for j in range(NHI[g] if "seg" not in ablate else 0):
                            nc.tensor.matmul(
                                out=aggT_ps[:],
                                lhsT=G_hi[:, int(hi_choff[g] - hi_choff[g0]) + j, :],
                                rhs=S[:, ch_off[g] - ch_off[g0] + k, :],
                                start=(k == 0), stop=(k == nmm - 1),
                            )
                            k += 1
                        aggT = small.tile([128, D], bf16, tag="aggTsb")
                        nc.vector.tensor_copy(out=aggT[:], in_=aggT_ps[:])
                        h_ps = psC.tile([128, D], f32, space="PSUM", tag="hps")
                        nc.tensor.matmul(
                            out=h_ps[:], lhsT=aggT[:], rhs=Ws[:, l, :],
                            start=True, stop=True,
                        )
                        emit_h_block(g, h_ps, l + 1)
    nc.compile()
    return nc


def kernel(feat, src, dst, W0, Ws):
    from concourse.bass_utils import run_bass_kernel_spmd

    plan, slabs = _prepare(feat, src, dst)
    nc = _build(plan)

    W0_np = np.asarray(W0, dtype=np.float32)
    Ws_np = np.transpose(np.asarray(Ws, dtype=np.float32), (1, 0, 2)).astype(
        ml_dtypes.bfloat16
    )  # [fi, layer, fo]
    in_maps = [
        {
            "idx_lo": slabs["idx_lo"][c],
            "idx_hi": slabs["idx_hi"][c],
            "dstrel": slabs["dstrel"][c],
            "ivedge": slabs["ivedge"][c],
            "featT": slabs["featT"][c],
            "W0": W0_np,
            "Ws": Ws_np,
        }
        for c in range(NCORES)
    ]
    res = None
    last_err = None
    for attempt in range(3):
        try:
            res = run_bass_kernel_spmd(nc, in_maps, core_ids=list(range(NCORES)))
            break
        except Exception as e:  # transient device hiccups (axon RPC, NRT recovery)
            last_err = e
            import time as _time

            _time.sleep(5)
    if res is None:
        raise last_err
    _RUNTIME["nc"] = nc
    _RUNTIME["in_maps"] = in_maps

    out = np.empty((N_NODES, (NLAYERS + 1) * D), dtype=np.float32)
    for c in range(NCORES):
        out[c * NPC : (c + 1) * NPC] = res.results[c]["out"][:NPC]
    return out

